# revision 7
# baseline (speedup 1.0000x reference)
"""Distributed exact inner-product top-k (brute-force kNN) on 8 TRN2 NeuronCores.

Sharding: codebook W is split row-wise into 8 shards of 25000 (one per core);
x is replicated.  Host pre-transposes both to bf16: the PE streams 1 output
column/cycle regardless of dtype at contraction 128 (fp8 DoubleRow only
pays off at contraction 256 - measured), so bf16 is free accuracy.

Device kernel (SPMD, identical graph per core, no collectives):
  - the vocab shard is processed in 12 bands of 2048 cols (+ a 424 tail);
    each band runs all 8 batch groups before moving on, so compute only
    ever waits for the first ~0.5MB of the W load instead of all 6.4MB
  - per 1024-col region (2 PSUM banks, 4 in flight): 2x bf16 matmuls
    [128 contraction, 512 cols] into PSUM (f32)
  - each region is drained by one of the only two engines that can read
    PSUM, statically balanced by their clocks and overheads:
      D: DVE windowed tensor_reduce(max) w=4 -> bf16 window maxima
         (even region of each band)
      A: Act copy PSUM -> fp8e4 raw scores, window-1 (odd region + tail);
         each Act copy lands in its own small tile and is DMA'd immediately
  - per-row outputs: 2872 bf16 w4-maxima + 12712 fp8 raw scores, streaming
    out on the SP and gpsimd DMA queues

Host merge (the all-gather + final top-k of the distributed ANN pattern):
  - per row, select every window whose (value + its route's EPS) clears
    (128th-largest window value - EPSMAX - slack); gather member columns
  - exact f64 re-rank of the candidates; final top-128 ordered like
    jax.lax.top_k (value desc, index asc)
  - exactness guard: containment holds if |device value - exact window max|
    <= EPS_route for every window that can matter; all such windows are
    selected, EPS is validated on them per-run, and violating rows
    (expected none) are recomputed exactly.  bf16 inputs keep the gap tiny
    (bf16-out windows ~0.25, fp8e4-out windows ~2.2 at |s|~45), so margins
    and candidate counts stay small and the host merge is cheap.
"""

import numpy as np

B = 1024
D = 128
VOCAB = 200000
NCORES = 8
VSHARD = VOCAB // NCORES  # 25000
REG = 1024  # 2 PSUM banks of f32
NBAND = 12  # bands of 2 regions (D, A)
TAIL = VSHARD - NBAND * 2 * REG  # 424 -> Act
TOPK = 128
NGRP = B // 128

# Region schedule per band: even region -> DVE w4 reduce ("D"), odd region
# -> Act fp8 copy ("A"); the 424-col tail -> Act.  This lands near the
# 0.96/1.2 GHz drain balance (DVE 12288 cols : Act 12712 per 25000).
W4_SEGS = [(2 * b * REG, REG) for b in range(NBAND)]
W1_SEGS = [((2 * b + 1) * REG, REG) for b in range(NBAND)] + [
    (2 * NBAND * REG, TAIL)
]
NW4 = sum(n // 4 for _, n in W4_SEGS)  # 3072
NW1 = sum(n for _, n in W1_SEGS)  # 12712
W4_OFF = np.concatenate([[0], np.cumsum([n // 4 for _, n in W4_SEGS])])
W1_OFF = np.concatenate([[0], np.cumsum([n for _, n in W1_SEGS])])

# |device window value - exact window max| bounds, validated at runtime:
# bf16 input quantization noise on x and W (~0.1-0.2) plus output
# quantization (bf16 ~0.2 for route D, fp8e4 ~2.2 for route A at |s|~45).
EPS4 = 0.45
EPS1 = 2.5
EPSMAX = EPS1
SLACK = 0.3

LAST_RESULTS = None  # BassKernelResults of the most recent run (for profiling)
_CACHED_NC = None


def build_kernel():
    import concourse.bass as bass  # noqa: F401
    import concourse.tile as tile
    from concourse import bacc, mybir

    F32 = mybir.dt.float32
    BF16 = mybir.dt.bfloat16
    FP8 = mybir.dt.float8e4
    AX = mybir.AxisListType.X
    MAX = mybir.AluOpType.max
    COPY = mybir.ActivationFunctionType.Copy

    nc = bacc.Bacc("TRN2", target_bir_lowering=False, debug=False)
    wt_d = nc.dram_tensor("wt", [D, VSHARD], BF16, kind="ExternalInput")
    xt_d = nc.dram_tensor("xt", [D, B], BF16, kind="ExternalInput")
    out4_d = nc.dram_tensor("out_w4", [B, NW4], BF16, kind="ExternalOutput")
    out1_d = nc.dram_tensor("out_w1", [B, NW1], FP8, kind="ExternalOutput")

    with tile.TileContext(nc) as tc:
        with (
            tc.tile_pool(name="wt", bufs=1) as wt_pool,
            tc.tile_pool(name="xt", bufs=1) as xt_pool,
            tc.tile_pool(name="psum", bufs=4, space="PSUM") as psum_pool,
            tc.tile_pool(name="out4", bufs=1) as out4_pool,
            tc.tile_pool(name="out1", bufs=6) as out1_pool,
        ):
            wt_sb = wt_pool.tile([D, VSHARD], BF16)
            xt_sb = xt_pool.tile([D, B], BF16)
            # xt first on SP, first W slab concurrently on the gpsimd queue:
            # the first band only needs ~0.5MB of W, so compute starts
            # almost immediately while the rest of W streams in.
            nc.sync.dma_start(xt_sb[:], xt_d[:])
            slabs = [512] * 8 + [1024] * 20 + [424]
            assert sum(slabs) == VSHARD
            lo = 0
            for s, w in enumerate(slabs):
                eng = nc.gpsimd if s % 2 == 0 else nc.sync
                eng.dma_start(wt_sb[:, lo:lo + w], wt_d[:, lo:lo + w])
                lo += w

            # per-group w4 accumulation tiles live across all bands
            out4_sbs = [
                out4_pool.tile([128, NW4], BF16, name=f"out4_{g}",
                               tag=f"out4_{g}")
                for g in range(NGRP)
            ]

            def do_region(g, base, w_cols, route, o_lo):
                xg = xt_sb[:, g * 128:(g + 1) * 128]
                ps = psum_pool.tile([128, REG], F32)
                for k in range(0, w_cols, 512):
                    kw = min(512, w_cols - k)
                    nc.tensor.matmul(
                        ps[:, k:k + kw],
                        xg,
                        wt_sb[:, base + k:base + k + kw],
                        start=True, stop=True,
                    )
                if route == "D":
                    nc.vector.tensor_reduce(
                        out4_sbs[g][:, o_lo:o_lo + w_cols // 4],
                        ps[:, :w_cols].rearrange("p (n w) -> p n w", w=4),
                        axis=AX, op=MAX,
                    )
                else:
                    o1 = out1_pool.tile([128, REG], FP8, tag="o1")
                    nc.scalar.activation(o1[:, :w_cols], ps[:, :w_cols], COPY)
                    eng = nc.sync if g % 2 == 0 else nc.gpsimd
                    eng.dma_start(
                        out1_d[g * 128:(g + 1) * 128, o_lo:o_lo + w_cols],
                        o1[:, :w_cols],
                    )

            for b in range(NBAND):
                for g in range(NGRP):
                    do_region(g, (2 * b) * REG, REG, "D", int(W4_OFF[b]))
                    do_region(g, (2 * b + 1) * REG, REG, "A", int(W1_OFF[b]))
                    if b == 6:  # first half of w4 windows is complete
                        eng = nc.sync if g % 2 == 0 else nc.gpsimd
                        eng.dma_start(
                            out4_d[g * 128:(g + 1) * 128, :int(W4_OFF[7])],
                            out4_sbs[g][:, :int(W4_OFF[7])],
                        )
            # tail region -> Act
            for g in range(NGRP):
                do_region(g, 2 * NBAND * REG, TAIL, "A", int(W1_OFF[NBAND]))
                eng = nc.sync if g % 2 == 0 else nc.gpsimd
                eng.dma_start(
                    out4_d[g * 128:(g + 1) * 128, int(W4_OFF[7]):],
                    out4_sbs[g][:, int(W4_OFF[7]):],
                )
    nc.compile()
    return nc


def _build_maps():
    """Per-window candidate columns and EPS.

    Returns (colmap [NWIN, 4] int64 with -1 pads, eps [NWIN] f32) where
    window order is [all w4 windows, all w1 windows] per core.
    """
    nwin = NW4 + NW1
    cm = np.full((nwin, 4), -1, np.int64)
    eps = np.empty(nwin, np.float32)
    for si, (lo, n) in enumerate(W4_SEGS):
        o = int(W4_OFF[si])
        j = np.arange(n // 4)[:, None]
        cm[o:o + n // 4] = lo + 4 * j + np.arange(4)[None, :]
    eps[:NW4] = EPS4
    for si, (lo, n) in enumerate(W1_SEGS):
        o = NW4 + int(W1_OFF[si])
        cm[o:o + n, 0] = lo + np.arange(n)
    eps[NW4:] = EPS1
    return cm, eps


_COLMAP, _WEPS = _build_maps()


def _topk_rows(vals, gidx, k):
    """Per-row top-k ordered like jax.lax.top_k: value desc, index asc."""
    order = np.lexsort((gidx, -vals), axis=-1)[:, :k]
    return (
        np.take_along_axis(gidx, order, axis=1),
        np.take_along_axis(vals, order, axis=1),
    )


def kernel(x: np.ndarray, W: np.ndarray, topk) -> np.ndarray:
    global LAST_RESULTS, _CACHED_NC
    import os

    import ml_dtypes

    from concourse.bass_utils import run_bass_kernel_spmd

    assert x.shape == (B, D) and W.shape == (VOCAB, D)
    assert int(topk) == TOPK
    x = np.ascontiguousarray(np.asarray(x, dtype=np.float32))
    W = np.ascontiguousarray(np.asarray(W, dtype=np.float32))

    if _CACHED_NC is None:
        _CACHED_NC = build_kernel()
    nc = _CACHED_NC

    xt = np.ascontiguousarray(x.T).astype(ml_dtypes.bfloat16)
    in_maps = []
    for i in range(NCORES):
        wt_i = np.ascontiguousarray(
            W[i * VSHARD:(i + 1) * VSHARD].T
        ).astype(ml_dtypes.bfloat16)
        in_maps.append({"wt": wt_i, "xt": xt})

    LAST_RESULTS = run_bass_kernel_spmd(
        nc,
        in_maps,
        core_ids=list(range(NCORES)),
        trace=bool(int(os.environ.get("KERNEL_TRACE", "0"))),
    )
    results = LAST_RESULTS.results

    # [B, 8*(NW4+NW1)] device window values, f32
    nwin = NW4 + NW1
    wm = np.empty((B, NCORES * nwin), np.float32)
    for i in range(NCORES):
        wm[:, i * nwin:i * nwin + NW4] = np.asarray(
            results[i]["out_w4"]).astype(np.float32)
        wm[:, i * nwin + NW4:(i + 1) * nwin] = np.asarray(
            results[i]["out_w1"]).astype(np.float32)
    nwin_all = NCORES * nwin
    weps_all = np.tile(_WEPS, NCORES)

    # Per-row selection on adjusted values v' = v + eps_w:
    # keep windows with v' >= kth_dev - EPSMAX - SLACK.
    wma = wm + weps_all[None, :]
    kth = np.partition(wm, nwin_all - TOPK, axis=1)[:, nwin_all - TOPK]
    tau = kth - EPSMAX - SLACK
    counts = (wma >= tau[:, None]).sum(axis=1)
    K = int(min(max(int(counts.max()), TOPK + 64), 4096))
    topw = np.argpartition(-wma, K - 1, axis=1)[:, :K]  # [B, K] window ids

    core_id = topw // nwin
    wi = topw % nwin
    cols = _COLMAP[wi]  # [B, K, 4], -1 = pad
    pad = cols < 0
    cand = (np.where(pad, 0, cols) + core_id[..., None] * VSHARD).reshape(B, K * 4)

    # Exact f64 re-rank of the candidate columns (pads scored -inf).
    x64 = x.astype(np.float64)
    W64 = W.astype(np.float64)
    exact = np.empty((B, K * 4), np.float64)
    STEP = 64
    for r0 in range(0, B, STEP):
        r1 = r0 + STEP
        gW = W64[cand[r0:r1]]  # [STEP, K*4, D]
        exact[r0:r1] = np.einsum("bjd,bd->bj", gW, x64[r0:r1])
    exact[pad.reshape(B, K * 4)] = -np.inf

    gidx_top, vals_top = _topk_rows(exact, cand, TOPK)

    # Exactness guards: EPS must hold on every selected window (any window
    # that can contain a true top-128 column is selected), and the
    # selection count must fit in K.
    dev_w = np.take_along_axis(wm, topw, axis=1)
    true_w = exact.reshape(B, K, 4).max(axis=2)
    werr = np.abs(dev_w - true_w)
    sel_eps = weps_all[topw]
    err_excess = (werr - sel_eps).max(axis=1)
    bad = (err_excess > 0) | (counts > K)
    if os.environ.get("KERNEL_DEBUG"):
        w4mask = (topw % nwin) < NW4
        e4 = werr[w4mask].max() if w4mask.any() else 0.0
        e1 = werr[~w4mask].max() if (~w4mask).any() else 0.0
        print(f"[kernel] K={K} counts max={counts.max()} "
              f"err4 max={e4:.3f} err1 max={e1:.3f} bad rows={int(bad.sum())}")
    for r in np.flatnonzero(bad):
        s = x64[r] @ W64.T
        gidx_top[r] = np.lexsort((np.arange(VOCAB), -s))[:TOPK]

    return gidx_top.astype(np.int32)
